# revision 1
# baseline (speedup 1.0000x reference)
"""Trainium2 Bass kernel for scatter(w_est -> W[rows, cols]) followed by X @ W.

Strategy (data-parallel, 8 NeuronCores):
  - Host: scatter w_est into W (256x256) - tiny, and numpy assignment matches
    the reference's last-write-wins scatter semantics.
  - Host: shard X row-wise into 8 shards of 62500 rows; transpose each shard
    to feature-major [256, 62500] (the TensorE contracts over the partition
    dim, so X must arrive feature-major) and pad to 62592 = 489*128 columns.
  - Device (per core): out[128r, 256] = sum_k XT[k_chunk, rtile].T @ W[k_chunk, :]
    accumulated over the two 128-feature chunks in PSUM, copied to SBUF by
    the vector engine, DMA'd back row-major. No transpose needed on the way
    out.
  - Host: concatenate the 8 [62500, 256] shards.
"""

import numpy as np

N_ROWS = 500000
D = 256
N_CORES = 8
RPC = N_ROWS // N_CORES            # 62500 rows per core
TILE_M = 128                       # output rows per matmul group
N_TILES = (RPC + TILE_M - 1) // TILE_M  # 489
RPC_PAD = N_TILES * TILE_M         # 62592
CHUNK_TILES = 16                   # row tiles fetched per input DMA (8KB/partition)

_CACHE = {}
LAST_RESULT = None  # BassKernelResults of the most recent run (for profiling)


def _build():
    import concourse.bass as bass
    import concourse.tile as tile
    from concourse import bacc, mybir

    MM_DT = mybir.dt.float32

    nc = bacc.Bacc("TRN2", target_bir_lowering=False, debug=False,
                   num_devices=N_CORES)
    xt = nc.dram_tensor("xt", [D, RPC_PAD], MM_DT, kind="ExternalInput").ap()
    w = nc.dram_tensor("w", [D, D], MM_DT, kind="ExternalInput").ap()
    out = nc.dram_tensor("out", [RPC_PAD, D], mybir.dt.float32,
                         kind="ExternalOutput").ap()

    with tile.TileContext(nc) as tc:
        with tc.tile_pool(name="wpool", bufs=1) as wpool, \
             tc.tile_pool(name="xpool", bufs=3) as xpool, \
             tc.tile_pool(name="opool", bufs=6) as opool, \
             tc.psum_pool(name="pspool", bufs=6) as pspool:
            w0 = wpool.tile([128, D], MM_DT)
            nc.sync.dma_start(w0[:], w[0:128, :])
            w1 = wpool.tile([128, D], MM_DT)
            nc.sync.dma_start(w1[:], w[128:256, :])

            t0 = 0
            while t0 < N_TILES:
                nt = min(CHUNK_TILES, N_TILES - t0)
                c0, c1 = t0 * TILE_M, (t0 + nt) * TILE_M
                xa = xpool.tile([128, nt * TILE_M], MM_DT, tag="xa")
                nc.sync.dma_start(xa[:], xt[0:128, c0:c1])
                xb = xpool.tile([128, nt * TILE_M], MM_DT, tag="xb")
                nc.sync.dma_start(xb[:], xt[128:256, c0:c1])

                for t in range(nt):
                    ps = pspool.tile([128, D], mybir.dt.float32)
                    sl = slice(t * TILE_M, (t + 1) * TILE_M)
                    nc.tensor.matmul(ps[:], xa[:, sl], w0[:],
                                     start=True, stop=False)
                    nc.tensor.matmul(ps[:], xb[:, sl], w1[:],
                                     start=False, stop=True)
                    ob = opool.tile([128, D], mybir.dt.float32)
                    nc.vector.tensor_copy(ob[:], ps[:])
                    r0 = (t0 + t) * TILE_M
                    nc.scalar.dma_start(out[r0:r0 + TILE_M, :], ob[:])
                t0 += nt

    nc.compile()
    return nc


def kernel(X, w_est, rows, cols):
    global LAST_RESULT
    from concourse.bass_utils import run_bass_kernel_spmd

    X = np.ascontiguousarray(np.asarray(X, dtype=np.float32))
    w_est = np.asarray(w_est, dtype=np.float32)
    rows = np.asarray(rows)
    cols = np.asarray(cols)

    W = np.zeros((D, D), dtype=np.float32)
    W[rows, cols] = w_est  # last-write-wins, same as XLA scatter-set

    if "nc" not in _CACHE:
        _CACHE["nc"] = _build()
    nc = _CACHE["nc"]

    in_maps = []
    for c in range(N_CORES):
        shard = X[c * RPC:(c + 1) * RPC]          # [62500, 256] view
        xtp = np.zeros((D, RPC_PAD), dtype=np.float32)
        xtp[:, :RPC] = shard.T
        in_maps.append({"xt": xtp, "w": W})

    res = run_bass_kernel_spmd(nc, in_maps, core_ids=list(range(N_CORES)))
    LAST_RESULT = res
    return np.concatenate([r["out"][:RPC] for r in res.results], axis=0)
